# revision 8
# baseline (speedup 1.0000x reference)
"""CrossNonLocal2D kernel v3 for Trainium2, 8-way batch-parallel SPMD.

Design (per core = one batch element, CoreSim-guided):
  The exp() of the 4096x4096 logit matrix is the hard floor (~130us on the
  Activation engine at 1.2GHz); everything else is scheduled to hide under
  it.  Attention runs in NP passes over n-windows of PW columns with
  double-buffered ST PSUM tiles so ACT never waits on the PE:

    per m-tile t (128 keys):
      ST   = phi_t^T @ theta[:, win]   2 matmuls -> [128, PW] PSUM
      PT_t = exp(ST)                   1 ACT      -> [128, PW] bf16 SBUF
      dacc+= PT_t                      1 DVE add (bf16 2x mode)
      yacc+= gT_t^T @ PT_t             2 accumulating matmuls (PSUM)
    epilogue: d = partition_all_reduce(dacc) (GpSimd); y = yacc * (1/d);
      z = w_eff @ y + b_eff + x_this; 1 DMA out per pass.

  Convs feeding it are bf16 (inputs pre-cast on host), with f32 x_this kept
  for the residual.  BN + g/out biases are folded on the host.
"""

import os
import sys
import time

import numpy as np

for _p in ("/opt/trn_rl_repo",):
    if os.path.isdir(_p) and _p not in sys.path:
        sys.path.insert(0, _p)

import ml_dtypes  # noqa: E402
import concourse.bacc as bacc  # noqa: E402
import concourse.mybir as mybir  # noqa: E402
import concourse.tile as tile  # noqa: E402
from concourse.bass import ts  # noqa: E402
from concourse import bass_isa  # noqa: E402
from concourse.bass_utils import run_bass_kernel_spmd  # noqa: E402

B, C, HH, WW = 8, 256, 64, 64
N = HH * WW  # 4096
I = 128  # inter channels
NCORES = 8
BN_EPS = 1e-5
PW = 1024  # n-window width per pass
NP = N // PW  # 4 passes
MT = N // 128  # 32 m-tiles
WIDE_MM = False  # out >512 fp32 rejected by hw ISA check (s3d3_mm_num_elements)

f32 = mybir.dt.float32
bf16 = mybir.dt.bfloat16
EXP = mybir.ActivationFunctionType.Exp
CPY = mybir.ActivationFunctionType.Copy
ADD = mybir.AluOpType.add
MULT = mybir.AluOpType.mult
RADD = bass_isa.ReduceOp.add


def build_module(repeat: int = 1, hw_loop: int = 0, dyn_loop: bool = False):
    """repeat: python-unrolled body count. hw_loop: if >0, wrap the body in a
    tc.For_i hardware loop with that trip count. dyn_loop: wrap the body in a
    For_i whose trip count is read at runtime from the int32 input "nrep"
    (NEFF stays small, one module serves every repeat count)."""
    nc = bacc.Bacc("TRN2", target_bir_lowering=False, debug=False,
                   num_devices=NCORES)

    if dyn_loop:
        nrep_d = nc.dram_tensor("nrep", [1, 1], mybir.dt.int32,
                                kind="ExternalInput")
    xt32_d = nc.dram_tensor("xt32", [C, N], f32, kind="ExternalInput")
    xtb_d = nc.dram_tensor("xtb", [C, N], bf16, kind="ExternalInput")
    xob_d = nc.dram_tensor("xob", [C, N], bf16, kind="ExternalInput")
    thw_d = nc.dram_tensor("thwT", [C, I], bf16, kind="ExternalInput")
    phw_d = nc.dram_tensor("phwT", [C, I], bf16, kind="ExternalInput")
    gw_d = nc.dram_tensor("gwT", [C, I], bf16, kind="ExternalInput")
    weff_d = nc.dram_tensor("weffT", [I, C], bf16, kind="ExternalInput")
    tb_d = nc.dram_tensor("tb", [I, 1], f32, kind="ExternalInput")
    pb_d = nc.dram_tensor("pb", [I, 1], f32, kind="ExternalInput")
    beff_d = nc.dram_tensor("beff", [128, 2], f32, kind="ExternalInput")
    out_d = nc.dram_tensor("out", [C, N], f32, kind="ExternalOutput")

    xt32_v = xt32_d.ap().rearrange("(a p) n -> p a n", p=128)
    xtb_v = xtb_d.ap().rearrange("(a p) n -> p a n", p=128)
    xob_v = xob_d.ap().rearrange("(a p) n -> p a n", p=128)
    out_v = out_d.ap().rearrange("(a p) n -> p a n", p=128)

    with tile.TileContext(nc) as tc:
        with (
            tc.tile_pool(name="const", bufs=1) as constp,
            tc.tile_pool(name="xbig", bufs=1) as xbig,
            tc.tile_pool(name="conv", bufs=1) as convp,
            tc.tile_pool(name="ptp", bufs=6) as ptp,
            tc.tile_pool(name="small", bufs=3) as smp,
            tc.tile_pool(name="outp", bufs=3) as outp,
            tc.tile_pool(name="pst", bufs=2, space="PSUM") as ps_st,
            tc.tile_pool(name="ppv", bufs=2, space="PSUM") as ps_pv,
        ):
            # ---- weights / constants (loaded once) ----
            thw = constp.tile([128, 2, I], bf16, tag="thw")
            nc.sync.dma_start(out=thw,
                              in_=thw_d.ap().rearrange("(a p) i -> p a i", p=128))
            phw = constp.tile([128, 2, I], bf16, tag="phw")
            nc.sync.dma_start(out=phw,
                              in_=phw_d.ap().rearrange("(a p) i -> p a i", p=128))
            gw = constp.tile([128, 2, I], bf16, tag="gw")
            nc.sync.dma_start(out=gw,
                              in_=gw_d.ap().rearrange("(a p) i -> p a i", p=128))
            weff = constp.tile([128, 2, 128], bf16, tag="weff")
            nc.sync.dma_start(out=weff,
                              in_=weff_d.ap().rearrange("i (h c) -> i h c", h=2))
            tb = constp.tile([128, 1], f32, tag="tb")
            nc.sync.dma_start(out=tb, in_=tb_d.ap())
            pb = constp.tile([128, 1], f32, tag="pb")
            nc.sync.dma_start(out=pb, in_=pb_d.ap())
            beff = constp.tile([128, 2], f32, tag="beff")
            nc.sync.dma_start(out=beff, in_=beff_d.ap())

            def body():
                xob = xbig.tile([128, 2, N], bf16, tag="xob", name="xob")
                xtb = xbig.tile([128, 2, N], bf16, tag="xtb", name="xtb")
                xt32 = xbig.tile([128, 2, N], f32, tag="xt32", name="xt32")
                # chunked loads ordered by first use: xob h0 (phi/gT pair 0),
                # xtb h0 (theta pair 0), rest, xt32 (first read ~60us in)
                # first-quarter loads tiny so the conv chain starts ~1.6us in;
                # the rest as big chunks on parallel queues
                nc.sync.dma_start(out=xob[:, :, 0:1024],
                                  in_=xob_v[:, :, 0:1024])
                nc.sync.dma_start(out=xtb[:, :, 0:1024],
                                  in_=xtb_v[:, :, 0:1024])
                nc.sync.dma_start(out=xob[:, :, 1024:N],
                                  in_=xob_v[:, :, 1024:N])
                nc.sync.dma_start(out=xtb[:, :, 1024:N],
                                  in_=xtb_v[:, :, 1024:N])
                nc.sync.dma_start(out=xt32, in_=xt32_v)

                th_c = convp.tile([128, N], bf16, tag="th", name="th")
                ph_c = convp.tile([128, N], bf16, tag="ph", name="ph")
                gT = convp.tile([128, MT, 128], bf16, tag="gT", name="gT")

                def th_ph_chunkpair(w_t, b_t, src, dst, half):
                    pc = ps_st.tile([128, 2, 512], f32, tag="st")
                    for c in range(2):
                        j = 2 * half + c
                        for a in range(2):
                            nc.tensor.matmul(pc[:, c, :],
                                             lhsT=w_t[:, a, :],
                                             rhs=src[:, a, ts(j, 512)],
                                             start=(a == 0), stop=(a == 1))
                    nc.vector.tensor_scalar_add(
                        dst[:, ts(half, 1024)],
                        pc[:].rearrange("p a n -> p (a n)"), b_t[:])

                def gT_group(grp, on_act=True):
                    # pv pool is safe ONLY because all gT groups are emitted
                    # before the first PV accumulator allocation; an in-loop
                    # gT group here deadlocks pool rotation against the live
                    # accumulator
                    pg = ps_pv.tile([128, 2, 512], f32, tag="pv")
                    pgv = pg[:].rearrange("p a (k i) -> p (a k) i", i=128)
                    for k in range(8):
                        t = 8 * grp + k
                        for a in range(2):
                            nc.tensor.matmul(pgv[:, k, :],
                                             lhsT=xob[:, a, ts(t, 128)],
                                             rhs=gw[:, a, :],
                                             start=(a == 0), stop=(a == 1))
                    if on_act:
                        nc.scalar.activation(gT[:, ts(grp, 8), :], pgv[:], CPY)
                    else:
                        nc.vector.tensor_copy(gT[:, ts(grp, 8), :], pgv[:])

                # prologue: theta window 0 first, then phi/gT (needed
                # progressively by pass-0 m-loop), then remaining theta.
                # (Interleaving these into the m-loop was tried and LOST
                # ~7us in sim: the 2-slot st pool stalls the exp stream.)
                th_ph_chunkpair(thw, tb, xtb, th_c, 0)
                th_ph_chunkpair(phw, pb, xob, ph_c, 0)
                gT_group(0)
                th_ph_chunkpair(phw, pb, xob, ph_c, 1)
                gT_group(1)
                th_ph_chunkpair(phw, pb, xob, ph_c, 2)
                gT_group(2)
                th_ph_chunkpair(phw, pb, xob, ph_c, 3)
                gT_group(3)
                for half in range(1, 4):
                    th_ph_chunkpair(thw, tb, xtb, th_c, half)

                # ---- attention: NP passes over n-windows of PW, with the
                # epilogue of pass p-1 software-pipelined into pass p ----
                def epilogue_a(pvv, dacc, hold):
                    dsum = smp.tile([128, PW], f32, tag="dsum")
                    nc.gpsimd.partition_all_reduce(dsum[:], dacc[:], 128, RADD)
                    rcp = smp.tile([128, PW], f32, tag="rcp")
                    nc.vector.reciprocal(rcp[:], dsum[:])
                    yb = smp.tile([128, PW], bf16, tag="yb")
                    nc.vector.tensor_tensor(yb[:], pvv, rcp[:], op=MULT)
                    return yb

                def epilogue_b(yb, n0):
                    ot = ps_st.tile([128, 2, 512], f32, tag="st")
                    ot2 = ps_st.tile([128, 2, 512], f32, tag="st")
                    for h in range(2):
                        dst_ps = ot if h == 0 else ot2
                        for c in range(2):
                            nc.tensor.matmul(dst_ps[:, c, :],
                                             lhsT=weff[:, h, :],
                                             rhs=yb[:, ts(c, 512)],
                                             start=True, stop=True)
                    ob = outp.tile([128, 2, PW], f32, tag="ob")
                    for h in range(2):
                        src = (ot if h == 0 else ot2)[:].rearrange(
                            "p a n -> p (a n)")
                        nc.vector.scalar_tensor_tensor(
                            ob[:, h, :], src, beff[:, h:h + 1],
                            xt32[:, h, n0:n0 + PW], op0=ADD, op1=ADD)
                    nc.sync.dma_start(out=out_v[:, :, n0:n0 + PW], in_=ob[:])

                def epilogue_last(pvv, dacc, n0):
                    # final pass: nothing left to hide behind, so pipeline
                    # the epilogue itself in two 512-wide halves
                    for c in range(2):
                        sl = slice(512 * c, 512 * (c + 1))
                        dsum = smp.tile([128, 512], f32, tag="dsumh")
                        nc.gpsimd.partition_all_reduce(dsum[:], dacc[:, sl],
                                                       128, RADD)
                        rcp = smp.tile([128, 512], f32, tag="rcph")
                        nc.vector.reciprocal(rcp[:], dsum[:])
                        yb = smp.tile([128, 512], bf16, tag="ybh")
                        nc.vector.tensor_tensor(yb[:], pvv[:, sl], rcp[:],
                                                op=MULT)
                        ot = ps_st.tile([128, 2, 512], f32, tag="st")
                        for h in range(2):
                            nc.tensor.matmul(ot[:, h, :], lhsT=weff[:, h, :],
                                             rhs=yb[:], start=True, stop=True)
                        ob = outp.tile([128, 2, 512], f32, tag="obh")
                        for h in range(2):
                            nc.vector.scalar_tensor_tensor(
                                ob[:, h, :], ot[:, h, :], beff[:, h:h + 1],
                                xt32[:, h, n0 + 512 * c:n0 + 512 * (c + 1)],
                                op0=ADD, op1=ADD)
                        nc.sync.dma_start(
                            out=out_v[:, :, n0 + 512 * c:n0 + 512 * (c + 1)],
                            in_=ob[:])

                prev = None  # (pvv, dacc, n0) of previous pass
                yb_prev = None
                for p in range(NP):
                    n0 = p * PW
                    pvt = ps_pv.tile([128, 2, 512], f32, tag="pv")
                    pvv = pvt[:].rearrange("p a n -> p (a n)")
                    dacc = smp.tile([128, PW], bf16, tag="dacc")
                    for t in range(MT):
                        st_t = ps_st.tile([128, 2, 512], f32, tag="st")
                        stv = st_t[:].rearrange("p a n -> p (a n)")
                        if WIDE_MM:
                            nc.tensor.matmul(stv, lhsT=ph_c[:, ts(t, 128)],
                                             rhs=th_c[:, n0:n0 + PW],
                                             start=True, stop=True)
                        else:
                            for c in range(2):
                                nc.tensor.matmul(
                                    st_t[:, c, :],
                                    lhsT=ph_c[:, ts(t, 128)],
                                    rhs=th_c[:, n0 + 512 * c:n0 + 512 * (c + 1)],
                                    start=True, stop=True)
                        pt_t = ptp.tile([128, PW], bf16, tag="pt")
                        nc.scalar.activation(pt_t[:], stv, EXP)
                        if t == 0:
                            nc.vector.tensor_copy(dacc[:], pt_t[:])
                        else:
                            nc.vector.tensor_tensor(dacc[:], dacc[:], pt_t[:],
                                                    op=ADD)
                        if WIDE_MM:
                            nc.tensor.matmul(pvv, lhsT=gT[:, t, :],
                                             rhs=pt_t[:],
                                             start=(t == 0), stop=(t == MT - 1))
                        else:
                            for c in range(2):
                                nc.tensor.matmul(pvt[:, c, :],
                                                 lhsT=gT[:, t, :],
                                                 rhs=pt_t[:, ts(c, 512)],
                                                 start=(t == 0),
                                                 stop=(t == MT - 1))
                        if t == 1 and prev is not None:
                            yb_prev = epilogue_a(*prev[:2], None)
                        if t == 4 and prev is not None:
                            epilogue_b(yb_prev, prev[2])
                    prev = (pvv, dacc, n0)
                epilogue_last(*prev)

            if dyn_loop:
                import concourse.bass as bass_mod
                nrep_t = constp.tile([1, 1], mybir.dt.int32, tag="nrep")
                nc.sync.dma_start(out=nrep_t, in_=nrep_d.ap())
                regs = []
                for eng_t in mybir.ALL_ENGINES:
                    eng = nc.engines[eng_t]
                    r = eng.alloc_register(f"nrep_{eng_t.name}")
                    eng.reg_load(r, nrep_t[:])
                    regs.append(r)
                rep_v = nc.snap(bass_mod.RegisterHandles(regs), donate=True,
                                min_val=1, max_val=1 << 20)
                with tc.For_i(0, rep_v) as _i:
                    for _r in range(max(1, repeat)):
                        body()
            elif hw_loop > 0:
                with tc.For_i(0, hw_loop) as _i:
                    body()
            else:
                for _rep in range(repeat):
                    body()

    nc.compile()
    return nc


_CACHE: dict = {}


def _get_built(repeat: int = 1, hw_loop: int = 0, dyn_loop: bool = False):
    key = (repeat, hw_loop, dyn_loop)
    if key not in _CACHE:
        _CACHE[key] = build_module(repeat, hw_loop, dyn_loop)
    return _CACHE[key]


def _make_runner(nc, n_cores: int, donate: bool = True):
    """Cached sharded-jit runner (compiles the PJRT executable once)."""
    import jax
    from jax.experimental.shard_map import shard_map
    from jax.sharding import Mesh, PartitionSpec
    from concourse import bass2jax

    bass2jax.install_neuronx_cc_hook()
    partition_name = (nc.partition_id_tensor.name
                      if nc.partition_id_tensor else None)
    in_names, out_names, out_avals, zero_shapes = [], [], [], []
    for alloc in nc.m.functions[0].allocations:
        if not isinstance(alloc, mybir.MemoryLocationSet):
            continue
        name = alloc.memorylocations[0].name
        if alloc.kind == "ExternalInput":
            if name != partition_name:
                in_names.append(name)
        elif alloc.kind == "ExternalOutput":
            out_names.append(name)
            shape = tuple(alloc.tensor_shape)
            dtype = mybir.dt.np(alloc.dtype)
            out_avals.append(jax.core.ShapedArray(shape, dtype))
            zero_shapes.append((shape, dtype))
    n_params = len(in_names)
    n_outs = len(out_avals)
    bind_in_names = list(in_names) + list(out_names)
    if partition_name is not None:
        bind_in_names.append(partition_name)
    donate_ids = tuple(range(n_params, n_params + n_outs))

    def _body(*args):
        operands = list(args)
        if partition_name is not None:
            operands.append(bass2jax.partition_id_tensor())
        outs = bass2jax._bass_exec_p.bind(
            *operands,
            out_avals=tuple(out_avals),
            in_names=tuple(bind_in_names),
            out_names=tuple(out_names),
            lowering_input_output_aliases=(),
            sim_require_finite=True,
            sim_require_nnan=True,
            nc=nc,
        )
        return tuple(outs)

    devices = jax.devices()[:n_cores]
    mesh = Mesh(np.asarray(devices), ("core",))
    in_specs = (PartitionSpec("core"),) * (n_params + n_outs)
    out_specs = (PartitionSpec("core"),) * len(out_names)
    sharded = jax.jit(
        shard_map(_body, mesh=mesh, in_specs=in_specs, out_specs=out_specs,
                  check_rep=False),
        donate_argnums=(donate_ids if donate else ()),
        keep_unused=True)

    def concat_inputs(in_maps):
        per_core = [[np.asarray(m[nm]) for nm in in_names] for m in in_maps]
        return [
            np.concatenate([per_core[c][i] for c in range(n_cores)], axis=0)
            for i in range(n_params)
        ]

    def run_concat(concat_in):
        """concat_in: list of (possibly device-resident) concatenated arrays."""
        import jax as _jax
        concat_zeros = [np.zeros((n_cores * s[0], *s[1:]), d)
                        for (s, d) in zero_shapes]
        out_arrs = sharded(*concat_in, *concat_zeros)
        _jax.block_until_ready(out_arrs)
        return out_arrs

    def split_outputs(out_arrs):
        return [
            {nm: np.asarray(out_arrs[i]).reshape(n_cores, *out_avals[i].shape)[c]
             for i, nm in enumerate(out_names)}
            for c in range(n_cores)
        ]

    def run_maps(in_maps):
        return split_outputs(run_concat(concat_inputs(in_maps)))

    def run_ops(ops):
        """ops: full operand list (inputs + output zero buffers), may be
        device-resident. Returns raw output device arrays."""
        import jax as _jax
        out_arrs = sharded(*ops)
        _jax.block_until_ready(out_arrs)
        return out_arrs

    run_maps.concat_inputs = concat_inputs
    run_maps.run_concat = run_concat
    run_maps.split_outputs = split_outputs
    run_maps.run_ops = run_ops
    run_maps.zero_shapes = [( (n_cores * s[0], *s[1:]), d) for (s, d) in zero_shapes]
    run_maps.in_names = in_names
    return run_maps


_RUNNERS: dict = {}


def _get_runner(repeat: int = 1, hw_loop: int = 0, dyn_loop: bool = False,
                donate: bool = True):
    key = (repeat, hw_loop, dyn_loop, donate)
    if key not in _RUNNERS:
        _RUNNERS[key] = _make_runner(_get_built(repeat, hw_loop, dyn_loop),
                                     NCORES, donate)
    return _RUNNERS[key]


def prep_maps(inputs: dict) -> list[dict]:
    """Host-side precompute: fold BN + g/out biases, transpose weights,
    pre-cast activations to bf16."""
    f = lambda k: np.asarray(inputs[k], np.float32)
    x_this = f("x_this").reshape(B, C, N)
    x_other = f("x_other").reshape(B, C, N)
    theta_w, theta_b = f("theta_w"), f("theta_b")
    phi_w, phi_b = f("phi_w"), f("phi_b")
    g_w, g_b = f("g_w"), f("g_b")
    out_w, out_b = f("out_w"), f("out_b")
    gam, bet = f("bn_gamma"), f("bn_beta")
    mean, var = f("bn_mean"), f("bn_var")

    s = (gam / np.sqrt(var + BN_EPS)).astype(np.float32)  # [C]
    w_eff = (out_w * s[:, None]).astype(np.float32)  # [C, I]
    b_eff = (s * (out_w @ g_b + out_b - mean) + bet).astype(np.float32)  # [C]

    bf = ml_dtypes.bfloat16
    common = {
        "thwT": np.ascontiguousarray(theta_w.T).astype(bf),
        "phwT": np.ascontiguousarray(phi_w.T).astype(bf),
        "gwT": np.ascontiguousarray(g_w.T).astype(bf),
        "weffT": np.ascontiguousarray(w_eff.T).astype(bf),
        "tb": np.ascontiguousarray(theta_b[:, None]),
        "pb": np.ascontiguousarray(phi_b[:, None]),
        "beff": np.ascontiguousarray(b_eff.reshape(2, 128).T),
    }
    return [
        {"xt32": np.ascontiguousarray(x_this[b]),
         "xtb": np.ascontiguousarray(x_this[b]).astype(bf),
         "xob": np.ascontiguousarray(x_other[b]).astype(bf), **common}
        for b in range(B)
    ]


def run(inputs: dict, repeat: int = 1, hw_loop: int = 0, dyn_loop: bool = False,
        nrep: int = 1, time_it: bool = False):
    maps = prep_maps(inputs)
    if dyn_loop:
        for m in maps:
            m["nrep"] = np.full((1, 1), nrep, np.int32)
    try:
        runner = _get_runner(repeat, hw_loop, dyn_loop)
        t0 = time.time()
        results = runner(maps)
        wall = time.time() - t0
    except Exception:
        nc = _get_built(repeat, hw_loop, dyn_loop)
        t0 = time.time()
        results = run_bass_kernel_spmd(nc, maps, list(range(NCORES))).results
        wall = time.time() - t0
    out = np.stack([np.asarray(results[b]["out"], np.float32)
                    for b in range(B)])
    out = out.reshape(B, C, HH, WW)
    if time_it:
        return out, wall
    return out


def kernel(**inputs) -> np.ndarray:
    return run(inputs)


# revision 9
# speedup vs baseline: 1.1225x; 1.1225x over previous
"""CrossNonLocal2D kernel v3 for Trainium2, 8-way batch-parallel SPMD.

Design (per core = one batch element, CoreSim-guided):
  The exp() of the 4096x4096 logit matrix is the hard floor (~130us on the
  Activation engine at 1.2GHz); everything else is scheduled to hide under
  it.  Attention runs in NP passes over n-windows of PW columns with
  double-buffered ST PSUM tiles so ACT never waits on the PE:

    per m-tile t (128 keys):
      ST   = phi_t^T @ theta[:, win]   2 matmuls -> [128, PW] PSUM
      PT_t = exp(ST)                   1 ACT      -> [128, PW] bf16 SBUF
      dacc+= PT_t                      1 DVE add (bf16 2x mode)
      yacc+= gT_t^T @ PT_t             2 accumulating matmuls (PSUM)
    epilogue: d = partition_all_reduce(dacc) (GpSimd); y = yacc * (1/d);
      z = w_eff @ y + b_eff + x_this; 1 DMA out per pass.

  Convs feeding it are bf16 (inputs pre-cast on host), with f32 x_this kept
  for the residual.  BN + g/out biases are folded on the host.
"""

import os
import sys
import time

import numpy as np

for _p in ("/opt/trn_rl_repo",):
    if os.path.isdir(_p) and _p not in sys.path:
        sys.path.insert(0, _p)

import ml_dtypes  # noqa: E402
import concourse.bacc as bacc  # noqa: E402
import concourse.mybir as mybir  # noqa: E402
import concourse.tile as tile  # noqa: E402
from concourse.bass import ts  # noqa: E402
from concourse import bass_isa  # noqa: E402
from concourse.bass_utils import run_bass_kernel_spmd  # noqa: E402

B, C, HH, WW = 8, 256, 64, 64
N = HH * WW  # 4096
I = 128  # inter channels
NCORES = 8
BN_EPS = 1e-5
PW = 1024  # n-window width per pass
NP = N // PW  # 4 passes
MT = N // 128  # 32 m-tiles
WIDE_MM = False  # out >512 fp32 rejected by hw ISA check (s3d3_mm_num_elements)

f32 = mybir.dt.float32
bf16 = mybir.dt.bfloat16
EXP = mybir.ActivationFunctionType.Exp
CPY = mybir.ActivationFunctionType.Copy
ADD = mybir.AluOpType.add
MULT = mybir.AluOpType.mult
RADD = bass_isa.ReduceOp.add


def build_module(repeat: int = 1, hw_loop: int = 0, dyn_loop: bool = False):
    """repeat: python-unrolled body count. hw_loop: if >0, wrap the body in a
    tc.For_i hardware loop with that trip count. dyn_loop: wrap the body in a
    For_i whose trip count is read at runtime from the int32 input "nrep"
    (NEFF stays small, one module serves every repeat count)."""
    nc = bacc.Bacc("TRN2", target_bir_lowering=False, debug=False,
                   num_devices=NCORES)

    if dyn_loop:
        nrep_d = nc.dram_tensor("nrep", [1, 1], mybir.dt.int32,
                                kind="ExternalInput")
    xt32_d = nc.dram_tensor("xt32", [C, N], f32, kind="ExternalInput")
    xtb_d = nc.dram_tensor("xtb", [C, N], bf16, kind="ExternalInput")
    xob_d = nc.dram_tensor("xob", [C, N], bf16, kind="ExternalInput")
    thw_d = nc.dram_tensor("thwT", [C, I], bf16, kind="ExternalInput")
    phw_d = nc.dram_tensor("phwT", [C, I], bf16, kind="ExternalInput")
    gw_d = nc.dram_tensor("gwT", [C, I], bf16, kind="ExternalInput")
    weff_d = nc.dram_tensor("weffT", [I, C], bf16, kind="ExternalInput")
    tb_d = nc.dram_tensor("tb", [I, 1], f32, kind="ExternalInput")
    pb_d = nc.dram_tensor("pb", [I, 1], f32, kind="ExternalInput")
    beff_d = nc.dram_tensor("beff", [128, 2], f32, kind="ExternalInput")
    out_d = nc.dram_tensor("out", [C, N], f32, kind="ExternalOutput")

    xt32_v = xt32_d.ap().rearrange("(a p) n -> p a n", p=128)
    xtb_v = xtb_d.ap().rearrange("(a p) n -> p a n", p=128)
    xob_v = xob_d.ap().rearrange("(a p) n -> p a n", p=128)
    out_v = out_d.ap().rearrange("(a p) n -> p a n", p=128)

    with tile.TileContext(nc) as tc:
        with (
            tc.tile_pool(name="const", bufs=1) as constp,
            tc.tile_pool(name="xbig", bufs=1) as xbig,
            tc.tile_pool(name="conv", bufs=1) as convp,
            tc.tile_pool(name="ptp", bufs=5) as ptp,
            tc.tile_pool(name="small", bufs=3) as smp,
            tc.tile_pool(name="outp", bufs=3) as outp,
            tc.tile_pool(name="pst", bufs=2, space="PSUM") as ps_st,
            tc.tile_pool(name="ppv", bufs=2, space="PSUM") as ps_pv,
        ):
            # ---- weights / constants (loaded once) ----
            thw = constp.tile([128, 2, I], bf16, tag="thw")
            nc.sync.dma_start(out=thw,
                              in_=thw_d.ap().rearrange("(a p) i -> p a i", p=128))
            phw = constp.tile([128, 2, I], bf16, tag="phw")
            nc.sync.dma_start(out=phw,
                              in_=phw_d.ap().rearrange("(a p) i -> p a i", p=128))
            gw = constp.tile([128, 2, I], bf16, tag="gw")
            nc.sync.dma_start(out=gw,
                              in_=gw_d.ap().rearrange("(a p) i -> p a i", p=128))
            weff = constp.tile([128, 2, 128], bf16, tag="weff")
            nc.sync.dma_start(out=weff,
                              in_=weff_d.ap().rearrange("i (h c) -> i h c", h=2))
            tb = constp.tile([128, 1], f32, tag="tb")
            nc.sync.dma_start(out=tb, in_=tb_d.ap())
            pb = constp.tile([128, 1], f32, tag="pb")
            nc.sync.dma_start(out=pb, in_=pb_d.ap())
            beff = constp.tile([128, 2], f32, tag="beff")
            nc.sync.dma_start(out=beff, in_=beff_d.ap())

            def body():
                xob = xbig.tile([128, 2, N], bf16, tag="xob", name="xob")
                xtb = xbig.tile([128, 2, N], bf16, tag="xtb", name="xtb")
                xt32 = xbig.tile([128, 2, N], f32, tag="xt32", name="xt32")
                # chunked loads ordered by first use: xob h0 (phi/gT pair 0),
                # xtb h0 (theta pair 0), rest, xt32 (first read ~60us in)
                # first-quarter loads tiny so the conv chain starts ~1.6us in;
                # the rest as big chunks on parallel queues
                nc.sync.dma_start(out=xob[:, :, 0:1024],
                                  in_=xob_v[:, :, 0:1024])
                nc.sync.dma_start(out=xtb[:, :, 0:1024],
                                  in_=xtb_v[:, :, 0:1024])
                nc.sync.dma_start(out=xob[:, :, 1024:N],
                                  in_=xob_v[:, :, 1024:N])
                nc.sync.dma_start(out=xtb[:, :, 1024:N],
                                  in_=xtb_v[:, :, 1024:N])
                nc.sync.dma_start(out=xt32, in_=xt32_v)

                th_c = convp.tile([128, N], bf16, tag="th", name="th")
                ph_c = convp.tile([128, N], bf16, tag="ph", name="ph")
                gT = convp.tile([128, MT, 128], bf16, tag="gT", name="gT")

                def th_ph_chunkpair(w_t, b_t, src, dst, half):
                    pc = ps_st.tile([128, 2, 512], f32, tag="st")
                    for c in range(2):
                        j = 2 * half + c
                        for a in range(2):
                            nc.tensor.matmul(pc[:, c, :],
                                             lhsT=w_t[:, a, :],
                                             rhs=src[:, a, ts(j, 512)],
                                             start=(a == 0), stop=(a == 1))
                    nc.vector.tensor_scalar_add(
                        dst[:, ts(half, 1024)],
                        pc[:].rearrange("p a n -> p (a n)"), b_t[:])

                def gT_group(grp, on_act=True):
                    # pv pool is safe ONLY because all gT groups are emitted
                    # before the first PV accumulator allocation; an in-loop
                    # gT group here deadlocks pool rotation against the live
                    # accumulator
                    pg = ps_pv.tile([128, 2, 512], f32, tag="pv")
                    pgv = pg[:].rearrange("p a (k i) -> p (a k) i", i=128)
                    for k in range(8):
                        t = 8 * grp + k
                        for a in range(2):
                            nc.tensor.matmul(pgv[:, k, :],
                                             lhsT=xob[:, a, ts(t, 128)],
                                             rhs=gw[:, a, :],
                                             start=(a == 0), stop=(a == 1))
                    if on_act:
                        nc.scalar.activation(gT[:, ts(grp, 8), :], pgv[:], CPY)
                    else:
                        nc.vector.tensor_copy(gT[:, ts(grp, 8), :], pgv[:])

                # prologue: theta window 0 first, then phi/gT (needed
                # progressively by pass-0 m-loop), then remaining theta.
                # (Interleaving these into the m-loop was tried and LOST
                # ~7us in sim: the 2-slot st pool stalls the exp stream.)
                th_ph_chunkpair(thw, tb, xtb, th_c, 0)
                th_ph_chunkpair(phw, pb, xob, ph_c, 0)
                gT_group(0)
                th_ph_chunkpair(phw, pb, xob, ph_c, 1)
                gT_group(1)
                th_ph_chunkpair(phw, pb, xob, ph_c, 2)
                gT_group(2)
                th_ph_chunkpair(phw, pb, xob, ph_c, 3)
                gT_group(3)
                for half in range(1, 4):
                    th_ph_chunkpair(thw, tb, xtb, th_c, half)

                # ---- attention: NP passes over n-windows of PW, with the
                # epilogue of pass p-1 software-pipelined into pass p ----
                def epilogue_a(pvv, dacc, hold):
                    dsum = smp.tile([128, PW], f32, tag="dsum")
                    nc.gpsimd.partition_all_reduce(dsum[:], dacc[:], 128, RADD)
                    rcp = smp.tile([128, PW], f32, tag="rcp")
                    nc.vector.reciprocal(rcp[:], dsum[:])
                    yb = smp.tile([128, PW], bf16, tag="yb")
                    nc.vector.tensor_tensor(yb[:], pvv, rcp[:], op=MULT)
                    return yb

                def epilogue_b(yb, n0):
                    ot = ps_st.tile([128, 2, 512], f32, tag="st")
                    ot2 = ps_st.tile([128, 2, 512], f32, tag="st")
                    for h in range(2):
                        dst_ps = ot if h == 0 else ot2
                        for c in range(2):
                            nc.tensor.matmul(dst_ps[:, c, :],
                                             lhsT=weff[:, h, :],
                                             rhs=yb[:, ts(c, 512)],
                                             start=True, stop=True)
                    ob = outp.tile([128, 2, PW], f32, tag="ob")
                    for h in range(2):
                        src = (ot if h == 0 else ot2)[:].rearrange(
                            "p a n -> p (a n)")
                        nc.vector.scalar_tensor_tensor(
                            ob[:, h, :], src, beff[:, h:h + 1],
                            xt32[:, h, n0:n0 + PW], op0=ADD, op1=ADD)
                    nc.sync.dma_start(out=out_v[:, :, n0:n0 + PW], in_=ob[:])

                def epilogue_last(pvv, dacc, n0):
                    # final pass: nothing left to hide behind, so pipeline
                    # the epilogue itself in two 512-wide halves
                    for c in range(2):
                        sl = slice(512 * c, 512 * (c + 1))
                        dsum = smp.tile([128, 512], f32, tag="dsumh")
                        nc.gpsimd.partition_all_reduce(dsum[:], dacc[:, sl],
                                                       128, RADD)
                        rcp = smp.tile([128, 512], f32, tag="rcph")
                        nc.vector.reciprocal(rcp[:], dsum[:])
                        yb = smp.tile([128, 512], bf16, tag="ybh")
                        nc.vector.tensor_tensor(yb[:], pvv[:, sl], rcp[:],
                                                op=MULT)
                        ot = ps_st.tile([128, 2, 512], f32, tag="st")
                        for h in range(2):
                            nc.tensor.matmul(ot[:, h, :], lhsT=weff[:, h, :],
                                             rhs=yb[:], start=True, stop=True)
                        ob = outp.tile([128, 2, 512], f32, tag="obh")
                        for h in range(2):
                            nc.vector.scalar_tensor_tensor(
                                ob[:, h, :], ot[:, h, :], beff[:, h:h + 1],
                                xt32[:, h, n0 + 512 * c:n0 + 512 * (c + 1)],
                                op0=ADD, op1=ADD)
                        nc.sync.dma_start(
                            out=out_v[:, :, n0 + 512 * c:n0 + 512 * (c + 1)],
                            in_=ob[:])

                prev = None  # (pvv, dacc, n0) of previous pass
                yb_prev = None
                for p in range(NP):
                    n0 = p * PW
                    pvt = ps_pv.tile([128, 2, 512], f32, tag="pv")
                    pvv = pvt[:].rearrange("p a n -> p (a n)")
                    dacc = smp.tile([128, PW], bf16, tag="dacc")
                    for t in range(MT):
                        st_t = ps_st.tile([128, 2, 512], f32, tag="st")
                        stv = st_t[:].rearrange("p a n -> p (a n)")
                        if WIDE_MM:
                            nc.tensor.matmul(stv, lhsT=ph_c[:, ts(t, 128)],
                                             rhs=th_c[:, n0:n0 + PW],
                                             start=True, stop=True)
                        else:
                            for c in range(2):
                                nc.tensor.matmul(
                                    st_t[:, c, :],
                                    lhsT=ph_c[:, ts(t, 128)],
                                    rhs=th_c[:, n0 + 512 * c:n0 + 512 * (c + 1)],
                                    start=True, stop=True)
                        pt_t = ptp.tile([128, PW], bf16, tag="pt")
                        nc.scalar.activation(pt_t[:], stv, EXP)
                        if t == 0:
                            nc.vector.tensor_copy(dacc[:], pt_t[:])
                        else:
                            nc.vector.tensor_tensor(dacc[:], dacc[:], pt_t[:],
                                                    op=ADD)
                        if WIDE_MM:
                            nc.tensor.matmul(pvv, lhsT=gT[:, t, :],
                                             rhs=pt_t[:],
                                             start=(t == 0), stop=(t == MT - 1))
                        else:
                            for c in range(2):
                                nc.tensor.matmul(pvt[:, c, :],
                                                 lhsT=gT[:, t, :],
                                                 rhs=pt_t[:, ts(c, 512)],
                                                 start=(t == 0),
                                                 stop=(t == MT - 1))
                        if t == 1 and prev is not None:
                            yb_prev = epilogue_a(*prev[:2], None)
                        if t == 4 and prev is not None:
                            epilogue_b(yb_prev, prev[2])
                    prev = (pvv, dacc, n0)
                epilogue_last(*prev)

            if dyn_loop:
                import concourse.bass as bass_mod
                nrep_t = constp.tile([1, 1], mybir.dt.int32, tag="nrep")
                nc.sync.dma_start(out=nrep_t, in_=nrep_d.ap())
                regs = []
                for eng_t in mybir.ALL_ENGINES:
                    eng = nc.engines[eng_t]
                    r = eng.alloc_register(f"nrep_{eng_t.name}")
                    eng.reg_load(r, nrep_t[:])
                    regs.append(r)
                rep_v = nc.snap(bass_mod.RegisterHandles(regs), donate=True,
                                min_val=1, max_val=1 << 20)
                with tc.For_i(0, rep_v) as _i:
                    for _r in range(max(1, repeat)):
                        body()
            elif hw_loop > 0:
                with tc.For_i(0, hw_loop) as _i:
                    body()
            else:
                for _rep in range(repeat):
                    body()

    nc.compile()
    return nc


_CACHE: dict = {}


def _get_built(repeat: int = 1, hw_loop: int = 0, dyn_loop: bool = False):
    key = (repeat, hw_loop, dyn_loop)
    if key not in _CACHE:
        _CACHE[key] = build_module(repeat, hw_loop, dyn_loop)
    return _CACHE[key]


def _make_runner(nc, n_cores: int, donate: bool = True):
    """Cached sharded-jit runner (compiles the PJRT executable once)."""
    import jax
    from jax.experimental.shard_map import shard_map
    from jax.sharding import Mesh, PartitionSpec
    from concourse import bass2jax

    bass2jax.install_neuronx_cc_hook()
    partition_name = (nc.partition_id_tensor.name
                      if nc.partition_id_tensor else None)
    in_names, out_names, out_avals, zero_shapes = [], [], [], []
    for alloc in nc.m.functions[0].allocations:
        if not isinstance(alloc, mybir.MemoryLocationSet):
            continue
        name = alloc.memorylocations[0].name
        if alloc.kind == "ExternalInput":
            if name != partition_name:
                in_names.append(name)
        elif alloc.kind == "ExternalOutput":
            out_names.append(name)
            shape = tuple(alloc.tensor_shape)
            dtype = mybir.dt.np(alloc.dtype)
            out_avals.append(jax.core.ShapedArray(shape, dtype))
            zero_shapes.append((shape, dtype))
    n_params = len(in_names)
    n_outs = len(out_avals)
    bind_in_names = list(in_names) + list(out_names)
    if partition_name is not None:
        bind_in_names.append(partition_name)
    donate_ids = tuple(range(n_params, n_params + n_outs))

    def _body(*args):
        operands = list(args)
        if partition_name is not None:
            operands.append(bass2jax.partition_id_tensor())
        outs = bass2jax._bass_exec_p.bind(
            *operands,
            out_avals=tuple(out_avals),
            in_names=tuple(bind_in_names),
            out_names=tuple(out_names),
            lowering_input_output_aliases=(),
            sim_require_finite=True,
            sim_require_nnan=True,
            nc=nc,
        )
        return tuple(outs)

    devices = jax.devices()[:n_cores]
    mesh = Mesh(np.asarray(devices), ("core",))
    in_specs = (PartitionSpec("core"),) * (n_params + n_outs)
    out_specs = (PartitionSpec("core"),) * len(out_names)
    sharded = jax.jit(
        shard_map(_body, mesh=mesh, in_specs=in_specs, out_specs=out_specs,
                  check_rep=False),
        donate_argnums=(donate_ids if donate else ()),
        keep_unused=True)

    def concat_inputs(in_maps):
        per_core = [[np.asarray(m[nm]) for nm in in_names] for m in in_maps]
        return [
            np.concatenate([per_core[c][i] for c in range(n_cores)], axis=0)
            for i in range(n_params)
        ]

    def run_concat(concat_in):
        """concat_in: list of (possibly device-resident) concatenated arrays."""
        import jax as _jax
        concat_zeros = [np.zeros((n_cores * s[0], *s[1:]), d)
                        for (s, d) in zero_shapes]
        out_arrs = sharded(*concat_in, *concat_zeros)
        _jax.block_until_ready(out_arrs)
        return out_arrs

    def split_outputs(out_arrs):
        return [
            {nm: np.asarray(out_arrs[i]).reshape(n_cores, *out_avals[i].shape)[c]
             for i, nm in enumerate(out_names)}
            for c in range(n_cores)
        ]

    def run_maps(in_maps):
        return split_outputs(run_concat(concat_inputs(in_maps)))

    def run_ops(ops):
        """ops: full operand list (inputs + output zero buffers), may be
        device-resident. Returns raw output device arrays."""
        import jax as _jax
        out_arrs = sharded(*ops)
        _jax.block_until_ready(out_arrs)
        return out_arrs

    run_maps.concat_inputs = concat_inputs
    run_maps.run_concat = run_concat
    run_maps.split_outputs = split_outputs
    run_maps.run_ops = run_ops
    run_maps.zero_shapes = [( (n_cores * s[0], *s[1:]), d) for (s, d) in zero_shapes]
    run_maps.in_names = in_names
    return run_maps


_RUNNERS: dict = {}


def _get_runner(repeat: int = 1, hw_loop: int = 0, dyn_loop: bool = False,
                donate: bool = True):
    key = (repeat, hw_loop, dyn_loop, donate)
    if key not in _RUNNERS:
        _RUNNERS[key] = _make_runner(_get_built(repeat, hw_loop, dyn_loop),
                                     NCORES, donate)
    return _RUNNERS[key]


def prep_maps(inputs: dict) -> list[dict]:
    """Host-side precompute: fold BN + g/out biases, transpose weights,
    pre-cast activations to bf16."""
    f = lambda k: np.asarray(inputs[k], np.float32)
    x_this = f("x_this").reshape(B, C, N)
    x_other = f("x_other").reshape(B, C, N)
    theta_w, theta_b = f("theta_w"), f("theta_b")
    phi_w, phi_b = f("phi_w"), f("phi_b")
    g_w, g_b = f("g_w"), f("g_b")
    out_w, out_b = f("out_w"), f("out_b")
    gam, bet = f("bn_gamma"), f("bn_beta")
    mean, var = f("bn_mean"), f("bn_var")

    s = (gam / np.sqrt(var + BN_EPS)).astype(np.float32)  # [C]
    w_eff = (out_w * s[:, None]).astype(np.float32)  # [C, I]
    b_eff = (s * (out_w @ g_b + out_b - mean) + bet).astype(np.float32)  # [C]

    bf = ml_dtypes.bfloat16
    common = {
        "thwT": np.ascontiguousarray(theta_w.T).astype(bf),
        "phwT": np.ascontiguousarray(phi_w.T).astype(bf),
        "gwT": np.ascontiguousarray(g_w.T).astype(bf),
        "weffT": np.ascontiguousarray(w_eff.T).astype(bf),
        "tb": np.ascontiguousarray(theta_b[:, None]),
        "pb": np.ascontiguousarray(phi_b[:, None]),
        "beff": np.ascontiguousarray(b_eff.reshape(2, 128).T),
    }
    return [
        {"xt32": np.ascontiguousarray(x_this[b]),
         "xtb": np.ascontiguousarray(x_this[b]).astype(bf),
         "xob": np.ascontiguousarray(x_other[b]).astype(bf), **common}
        for b in range(B)
    ]


def run(inputs: dict, repeat: int = 1, hw_loop: int = 0, dyn_loop: bool = False,
        nrep: int = 1, time_it: bool = False):
    maps = prep_maps(inputs)
    if dyn_loop:
        for m in maps:
            m["nrep"] = np.full((1, 1), nrep, np.int32)
    try:
        runner = _get_runner(repeat, hw_loop, dyn_loop)
        t0 = time.time()
        results = runner(maps)
        wall = time.time() - t0
    except Exception:
        nc = _get_built(repeat, hw_loop, dyn_loop)
        t0 = time.time()
        results = run_bass_kernel_spmd(nc, maps, list(range(NCORES))).results
        wall = time.time() - t0
    out = np.stack([np.asarray(results[b]["out"], np.float32)
                    for b in range(B)])
    out = out.reshape(B, C, HH, WW)
    if time_it:
        return out, wall
    return out


def kernel(**inputs) -> np.ndarray:
    return run(inputs)
